# revision 19
# baseline (speedup 1.0000x reference)
"""Trainium2 Bass kernel for nn_CNN_24472723653055 (AdderNet CNN), v2.

Data-parallel over 8 NeuronCores: 2 images per core. BN batch stats and the
global LayerNorm stats are synchronized with small AllReduces.

v2 algorithm — thermometer-quantized adder layers (L2, L3):
  |a - w| = (a - w) + 2*relu(w - a), and with the activation a quantized onto
  a grid t_0=0 < t_1 < ... < t_B ≈ max(w) (log-spaced), the telescoping
    relu(w - t_q) = sum_b 1[q <= b] * (relu(w - t_b) - relu(w - t_{b+1}))
  turns the nonlinear term into B matmuls:
    R[c,p] = sum_b sum_k DiffPlane_b[c,k] * T_b[k,p]
  where T_b[k,p] = 1[a_kp < mid_b] is a bf16 thermometer bit tile (one cheap
  DVE compare per (b, tap-view)) and DiffPlane_b = clip(w - t_b, 0, t_{b+1}-t_b)
  is host-precomputed and shipped as fp8e4m3 (values in [0, dt_b]).
  The matmul contracts (b, ci, tap) on partitions; output lands directly in
  [c, p] layout, so BN stats/apply are free-dim ops and no transposes remain.
  Linear corrections: acc = Sigma|a-w| = xs[p] + 2R[c,p] - wsum[c]; xs is fed
  into the same psum via a rank-1 (0.5*ones x xs) matmul.

Layer 1 (Cin=1, 6 taps) stays exact: pk[c,p] = x[p,tap] - w1[c,tap] via two
f32r rank-1 matmuls per (img,tap); Sigma_tap min(pk,0) accumulates on DVE;
|pk| = pk - 2*min(pk,0) folds the corrections into pxb = Sigma_tap bcast(x).
"""

import sys

sys.path.insert(0, "/opt/trn_rl_repo")

import numpy as np

N_CORES = 8
N_LOC = 2            # images per core
N_TOT = 16

C1, H1, W1 = 128, 196, 3
HO1 = 96
P1 = N_LOC * HO1 * W1          # 576
C2, HO2 = 256, 46
P2 = N_LOC * HO2 * W1          # 276
C3, HO3, WO3 = 384, 21, 2
P3 = N_LOC * HO3 * WO3         # 84
TAPS1, TAPS2 = 6, 6
TAPS3 = 12                     # (kh=6) x (kw=2)

B2 = 8                         # thermometer levels, layer 2
B3 = 6                         # thermometer levels, layer 3

EPS_BN = 1e-5
EPS_LN = 1e-5
EPS_L2 = 1e-12

_BUILD_CACHE = {}


def build_program(single=False):
    """single=True builds a 1-core variant with collectives replaced by
    DRAM copies — only for TimelineSim cost-model analysis."""
    import concourse.bass as bass
    import concourse.bacc as bacc
    import concourse.tile as tile
    import concourse.mybir as mybir

    dt = mybir.dt
    f32 = dt.float32
    f32r = dt.float32r
    bf16 = dt.bfloat16
    fp8 = dt.float8e4
    Alu = mybir.AluOpType
    Act = mybir.ActivationFunctionType

    nc = bacc.Bacc("TRN2", target_bir_lowering=False, debug=False,
                   num_devices=1 if single else N_CORES)

    # ------------------------------------------------------------------ I/O
    # x split hi/lo in bf16 so the L1 broadcast matmuls run at bf16 speed
    # with ~fp32 precision (x = xh + xl exactly to ~2^-16 rel).
    xh_in = nc.dram_tensor("xh_in", [1, N_LOC * H1 * W1], bf16, kind="ExternalInput").ap()
    xl_in = nc.dram_tensor("xl_in", [1, N_LOC * H1 * W1], bf16, kind="ExternalInput").ap()
    nw1t = nc.dram_tensor("nw1t", [1, TAPS1 * C1], bf16, kind="ExternalInput").ap()
    nw1s = nc.dram_tensor("nw1s", [128, 1], f32, kind="ExternalInput").ap()
    pl2_d = nc.dram_tensor("pl2", [128, B2 * TAPS2 * C2], fp8, kind="ExternalInput").ap()
    pl3_d = nc.dram_tensor("pl3", [128, B3 * 2 * TAPS3 * C3], fp8, kind="ExternalInput").ap()
    nws2_d = nc.dram_tensor("nws2", [128, 2], f32, kind="ExternalInput").ap()
    nws3_d = nc.dram_tensor("nws3", [128, 3], f32, kind="ExternalInput").ap()
    mid2_d = nc.dram_tensor("mid2", [128, B2], f32, kind="ExternalInput").ap()
    mid3_d = nc.dram_tensor("mid3", [128, B3], f32, kind="ExternalInput").ap()
    wfcp = nc.dram_tensor("wfcp", [128, 6 * 3 * 42], f32, kind="ExternalInput").ap()
    g1_d = nc.dram_tensor("g1_d", [C1], f32, kind="ExternalInput").ap()
    b1_d = nc.dram_tensor("b1_d", [C1], f32, kind="ExternalInput").ap()
    g2_d = nc.dram_tensor("g2_d", [C2], f32, kind="ExternalInput").ap()
    b2_d = nc.dram_tensor("b2_d", [C2], f32, kind="ExternalInput").ap()
    g3_d = nc.dram_tensor("g3_d", [C3], f32, kind="ExternalInput").ap()
    b3_d = nc.dram_tensor("b3_d", [C3], f32, kind="ExternalInput").ap()
    bfc_d = nc.dram_tensor("bfc_d", [6], f32, kind="ExternalInput").ap()
    out_d = nc.dram_tensor("out", [1, N_LOC * 6], f32, kind="ExternalOutput").ap()

    groups = [list(range(N_CORES))]
    HALF = HO1 * W1            # 288

    with tile.TileContext(nc) as tc:
        with tc.tile_pool(name="weights", bufs=1) as wp, \
             tc.tile_pool(name="acts", bufs=1) as ap_pool, \
             tc.tile_pool(name="consts", bufs=1) as cp, \
             tc.tile_pool(name="smalls", bufs=1) as sp, \
             tc.tile_pool(name="dram", bufs=1, space="DRAM") as dram:

            # ---------------------------------------------------- constants
            ones_row = cp.tile([1, 576], bf16)      # rank-1 lhsT/rhs rows
            nc.vector.memset(ones_row[:], 1.0)
            halves_row = cp.tile([1, 128], f32)     # 0.5-valued lhsT for xs feeds
            nc.vector.memset(halves_row[:], 0.5)
            ones_k = cp.tile([128, 1], f32)         # stats reduction lhsT
            nc.vector.memset(ones_k[:], 1.0)
            ones_kb = cp.tile([128, 1], bf16)       # colsum lhsT (bf16)
            nc.vector.memset(ones_kb[:], 1.0)

            # ---------------------------------------------------- input DMAs
            xh_sb = wp.tile([1, N_LOC * H1 * W1], bf16)
            nc.sync.dma_start(xh_sb[:], xh_in)
            xl_sb = wp.tile([1, N_LOC * H1 * W1], bf16)
            nc.sync.dma_start(xl_sb[:], xl_in)
            nw1_sb = wp.tile([1, TAPS1 * C1], bf16)
            nc.sync.dma_start(nw1_sb[:], nw1t)
            nw1s_sb = sp.tile([128, 1], f32)
            nc.sync.dma_start(nw1s_sb[:], nw1s)
            mid2_sb = sp.tile([128, B2], f32)
            nc.sync.dma_start(mid2_sb[:], mid2_d)
            mid3_sb = sp.tile([128, B3], f32)
            nc.sync.dma_start(mid3_sb[:], mid3_d)
            nws2_sb = sp.tile([128, 2], f32)
            nc.sync.dma_start(nws2_sb[:], nws2_d)
            nws3_sb = sp.tile([128, 3], f32)
            nc.sync.dma_start(nws3_sb[:], nws3_d)
            bfc_sb = sp.tile([1, 6], f32)
            nc.gpsimd.dma_start(bfc_sb[:], bfc_d.rearrange("(one j) -> one j", one=1))
            gb1 = sp.tile([128, 2], f32)
            nc.gpsimd.dma_start(gb1[:, 0:1], g1_d.rearrange("(p one) -> p one", one=1))
            nc.gpsimd.dma_start(gb1[:, 1:2], b1_d.rearrange("(p one) -> p one", one=1))
            gam2 = sp.tile([128, 2], f32)
            bet2 = sp.tile([128, 2], f32)
            gam3 = sp.tile([128, 3], f32)
            bet3 = sp.tile([128, 3], f32)
            for cb in range(2):
                nc.gpsimd.dma_start(gam2[:, cb:cb + 1],
                                    g2_d[cb * 128:(cb + 1) * 128].rearrange("(p one) -> p one", one=1))
                nc.gpsimd.dma_start(bet2[:, cb:cb + 1],
                                    b2_d[cb * 128:(cb + 1) * 128].rearrange("(p one) -> p one", one=1))
            for cb in range(3):
                nc.gpsimd.dma_start(gam3[:, cb:cb + 1],
                                    g3_d[cb * 128:(cb + 1) * 128].rearrange("(p one) -> p one", one=1))
                nc.gpsimd.dma_start(bet3[:, cb:cb + 1],
                                    b3_d[cb * 128:(cb + 1) * 128].rearrange("(p one) -> p one", one=1))
            wfc_sb = wp.tile([128, 6 * 3 * 42], f32)
            nc.sync.dma_start(wfc_sb[:], wfcp)
            # big fp8 plane tensors (per-b chunks so deps are fine-grained)
            pl2_sb = wp.tile([128, B2 * TAPS2 * C2], fp8)
            for b in range(B2):
                w = TAPS2 * C2
                eng = nc.sync if b % 2 == 0 else nc.scalar
                eng.dma_start(pl2_sb[:, b * w:(b + 1) * w],
                              pl2_d[:, b * w:(b + 1) * w])
            pl3_sb = wp.tile([128, B3 * 2 * TAPS3 * C3], fp8)
            for h in range(2 * B3):
                w = TAPS3 * C3
                eng = nc.sync if h % 2 == 0 else nc.scalar
                eng.dma_start(pl3_sb[:, h * w:(h + 1) * w],
                              pl3_d[:, h * w:(h + 1) * w])

            # persistent activation tensors
            acc1 = ap_pool.tile([128, P1], f32)        # layer1 Sigma|x-w|, [c1, p1]
            act1 = ap_pool.tile([128, P1], bf16)
            t2 = [ap_pool.tile([128, P1], bf16, name=f"t2_{b}") for b in range(B2)]
            y2 = [ap_pool.tile([128, P2], f32, name=f"y2_{cb}") for cb in range(2)]
            act2 = [ap_pool.tile([128, P2], bf16, name=f"act2_{cb}") for cb in range(2)]
            t3 = [[ap_pool.tile([128, P2], bf16, name=f"t3_{b}_{cb}") for cb in range(2)]
                  for b in range(B3)]
            y3 = [ap_pool.tile([128, P3], f32, name=f"y3_{cb}") for cb in range(3)]
            act3 = ap_pool.tile([128, 3 * P3], f32)    # [ci, (cb, n, howo)]
            cs1 = sp.tile([1, P1], f32)
            xs2 = sp.tile([1, P2], f32)
            cs2 = sp.tile([1, P2], f32)
            xs3 = sp.tile([1, P3], f32)

            # ---------------------------------------------------- helpers
            def allreduce(sbuf_src_aps, widths, name):
                total = sum(a.shape[0] * w for a, w in zip(sbuf_src_aps, widths))
                cin = dram.tile([1, total], f32, name=f"cc_in_{name}")
                cout = dram.tile([1, total], f32, name=f"cc_out_{name}")
                off = 0
                for a, w in zip(sbuf_src_aps, widths):
                    n = a.shape[0] * w
                    nc.sync.dma_start(
                        cin[0:1, off:off + n].rearrange("one (p w) -> (one p) w", w=w), a)
                    off += n
                if single:
                    nc.gpsimd.dma_start(cout[:], cin[:])
                else:
                    nc.gpsimd.collective_compute(
                        "AllReduce", Alu.add, replica_groups=groups,
                        ins=[cin.opt()], outs=[cout.opt()])
                return cout

            def bn_affine(st, gamma, beta, k, n_bn, name):
                """Vectorized over k channel-tiles at once. st [128, 2k] holds
                per-channel sums (cols 0:k) and sumsqs (cols k:2k). Returns
                scale/bias [128, k] for act = relu(scale*acc + bias) where the
                pre-BN value is -acc (acc = Sigma|a-w| >= 0)."""
                t_pool = sp
                mm = t_pool.tile([128, 2 * k], f32, name=f"{name}_mm")
                m2 = t_pool.tile([128, k], f32, name=f"{name}_m2")
                tv = t_pool.tile([128, k], f32, name=f"{name}_tv")
                s_ = t_pool.tile([128, k], f32, name=f"{name}_s")
                r0 = t_pool.tile([128, k], f32, name=f"{name}_r0")
                r0sq = t_pool.tile([128, k], f32, name=f"{name}_r0sq")
                av = t_pool.tile([128, k], f32, name=f"{name}_av")
                bv = t_pool.tile([128, k], f32, name=f"{name}_bv")
                rr = t_pool.tile([128, k], f32, name=f"{name}_rr")
                gr = t_pool.tile([128, k], f32, name=f"{name}_gr")
                bmt = t_pool.tile([128, k], f32, name=f"{name}_bmt")
                scale = t_pool.tile([128, k], f32, name=f"{name}_scale")
                bias = t_pool.tile([128, k], f32, name=f"{name}_bias")
                nc.vector.tensor_scalar(out=mm[:], in0=st, scalar1=1.0 / n_bn,
                                        scalar2=None, op0=Alu.mult)
                mean, msq = mm[:, 0:k], mm[:, k:2 * k]
                nc.vector.tensor_tensor(out=m2[:], in0=mean, in1=mean, op=Alu.mult)
                nc.vector.scalar_tensor_tensor(out=tv[:], in0=msq, scalar=EPS_BN,
                                               in1=m2[:], op0=Alu.add, op1=Alu.subtract)
                nc.scalar.activation(out=s_[:], in_=tv[:], func=Act.Sqrt)
                nc.vector.reciprocal(out=r0[:], in_=s_[:])
                # one Newton step for rsqrt accuracy: r = r0*(1.5 - 0.5*tv*r0^2)
                nc.vector.tensor_tensor(out=r0sq[:], in0=r0[:], in1=r0[:], op=Alu.mult)
                nc.vector.tensor_tensor(out=av[:], in0=tv[:], in1=r0sq[:], op=Alu.mult)
                nc.vector.tensor_scalar(out=bv[:], in0=av[:], scalar1=-0.5,
                                        scalar2=1.5, op0=Alu.mult, op1=Alu.add)
                nc.vector.tensor_tensor(out=rr[:], in0=r0[:], in1=bv[:], op=Alu.mult)
                nc.vector.tensor_tensor(out=gr[:], in0=gamma, in1=rr[:], op=Alu.mult)
                nc.vector.tensor_scalar(out=scale[:], in0=gr[:], scalar1=-1.0,
                                        scalar2=None, op0=Alu.mult)
                nc.vector.tensor_tensor(out=bmt[:], in0=gr[:], in1=mean, op=Alu.mult)
                nc.vector.tensor_tensor(out=bias[:], in0=bmt[:], in1=beta, op=Alu.add)
                return scale, bias

            # =================================================== layer 1
            xhv = xh_sb.rearrange("one (n h w) -> one n h w", n=N_LOC, h=H1, w=W1)
            xlv = xl_sb.rearrange("one (n h w) -> one n h w", n=N_LOC, h=H1, w=W1)
            accmin = ap_pool.tile([128, P1], f32)    # Sigma_tap min(pk, 0)  (<= 0)
            with tc.tile_pool(name="ps1", bufs=2, space="PSUM") as ps1, \
                 tc.tile_pool(name="pxbp", bufs=2, space="PSUM") as pxbp:
                for half in range(N_LOC):
                    pxb = pxbp.tile([128, HALF], f32, tag="pxb", name="pxb")
                    dst = accmin[:, half * HALF:(half + 1) * HALF]
                    for tap in range(TAPS1):
                        pk = ps1.tile([128, HALF], f32, tag="pk", name="pk")
                        xhrow = xhv[0:1, half, tap:tap + 2 * HO1 - 1:2, :]
                        xlrow = xlv[0:1, half, tap:tap + 2 * HO1 - 1:2, :]
                        nc.tensor.matmul(pk[:, :], lhsT=(ones_row[0:1, 0:128]),
                                         rhs=(xhrow), start=True, stop=False)
                        nc.tensor.matmul(pk[:, :], lhsT=(ones_row[0:1, 0:128]),
                                         rhs=(xlrow), start=False, stop=False)
                        nc.tensor.matmul(pk[:, :],
                                         lhsT=(nw1_sb[0:1, tap * C1:(tap + 1) * C1]),
                                         rhs=(ones_row[0:1, 0:HALF]),
                                         start=False, stop=True)
                        # pxb accumulates the raw x broadcast: Sigma_tap x[p,tap]
                        nc.tensor.matmul(pxb[:, :], lhsT=(ones_row[0:1, 0:128]),
                                         rhs=(xhrow), start=(tap == 0), stop=False)
                        nc.tensor.matmul(pxb[:, :], lhsT=(ones_row[0:1, 0:128]),
                                         rhs=(xlrow), start=False,
                                         stop=(tap == TAPS1 - 1))
                        if tap == 0:
                            nc.vector.tensor_scalar(out=dst, in0=pk[:, :], scalar1=0.0,
                                                    scalar2=None, op0=Alu.min)
                        else:
                            nc.vector.scalar_tensor_tensor(out=dst, in0=pk[:, :],
                                                           scalar=0.0, in1=dst,
                                                           op0=Alu.min, op1=Alu.add)
                    # acc1 = pxb - w1sum - 2*accmin   (= Sigma_tap |x - w1|)
                    asl = acc1[:, half * HALF:(half + 1) * HALF]
                    nc.vector.tensor_scalar(out=asl, in0=dst, scalar1=-2.0,
                                            scalar2=nw1s_sb[:], op0=Alu.mult,
                                            op1=Alu.add)
                    nc.vector.tensor_tensor(out=asl, in0=asl, in1=pxb[:, :], op=Alu.add)

            # BN1 stats (local): per-channel sum & sumsq over free dim
            s1 = sp.tile([128, 2], f32)
            scr1 = ap_pool.tile([128, P1], f32)
            nc.vector.tensor_scalar(out=scr1[:], in0=acc1[:], scalar1=0.0, scalar2=None,
                                    op0=Alu.add, op1=Alu.add, accum_out=s1[:, 0:1])
            nc.scalar.activation(out=scr1[:], in_=acc1[:], func=Act.Square,
                                 accum_out=s1[:, 1:2])
            cc1 = allreduce([s1[:, 0:2]], [2], "bn1")
            st1 = sp.tile([128, 2], f32)
            nc.sync.dma_start(st1[:, 0:2],
                              cc1[0:1, 0:256].rearrange("one (p w) -> (one p) w", w=2))
            sc1, bi1 = bn_affine(st1[:, 0:2], gb1[:, 0:1], gb1[:, 1:2], 1,
                                 N_TOT * HO1 * W1, "bn1")
            nc.scalar.activation(out=act1[:], in_=acc1[:], func=Act.Relu,
                                 scale=sc1[:], bias=bi1[:])
            # thermometer bits for layer 2: t2[b] = 1[act1 < mid2_b]
            for b in range(B2):
                nc.vector.tensor_scalar(out=t2[b][:], in0=act1[:],
                                        scalar1=mid2_sb[:, b:b + 1], scalar2=None,
                                        op0=Alu.is_lt)

            # =================================================== layer 2
            a1v = act1.rearrange("p (n h w) -> p n h w", n=N_LOC, h=HO1, w=W1)
            with tc.tile_pool(name="ps2", bufs=2, space="PSUM") as ps2, \
                 tc.tile_pool(name="csp", bufs=2, space="PSUM") as csp:
                # column sums of act1 -> cs1 [1, P1]; then xs2[p] = Sigma_tap cs1
                for half in range(N_LOC):
                    pcs = csp.tile([1, HALF], f32, tag="pcs", name="pcs")
                    nc.tensor.matmul(pcs[0:1, :], lhsT=ones_kb[:, 0:1],
                                     rhs=act1[:, half * HALF:(half + 1) * HALF],
                                     start=True, stop=True)
                    nc.vector.tensor_copy(cs1[0:1, half * HALF:(half + 1) * HALF],
                                          pcs[0:1, :])
                cs1v = cs1.rearrange("one (n h w) -> one n h w", n=N_LOC, h=HO1, w=W1)
                xs2v = xs2.rearrange("one (n h w) -> one n h w", n=N_LOC, h=HO2, w=W1)
                nc.vector.tensor_scalar(out=xs2v[:], in0=cs1v[0:1, :, 0:2 * HO2 - 1:2, :],
                                        scalar1=0.0, scalar2=None, op0=Alu.add)
                for tap in range(1, TAPS2):
                    nc.vector.tensor_tensor(out=xs2v[:], in0=xs2v[:],
                                            in1=cs1v[0:1, :, tap:tap + 2 * HO2 - 1:2, :],
                                            op=Alu.add)
                # main accumulation: psum2[ct] = R2 + 0.5*xs2
                st2_pool = sp
                for ct in range(2):
                    pt = ps2.tile([128, P2], f32, tag="pt2", name="pt2")
                    nc.tensor.matmul(pt[:, :], lhsT=halves_row[0:1, :],
                                     rhs=xs2[0:1, :], start=True, stop=False)
                    for b in range(B2):
                        tv2 = t2[b].rearrange("p (n h w) -> p n h w",
                                              n=N_LOC, h=HO1, w=W1)
                        for tap in range(TAPS2):
                            lix = (b * TAPS2 + tap) * C2 + ct * 128
                            nc.tensor.matmul(
                                pt[:, :],
                                lhsT=pl2_sb[:, lix:lix + 128],
                                rhs=tv2[:, :, tap:tap + 2 * HO2 - 1:2, :],
                                start=False,
                                stop=(b == B2 - 1 and tap == TAPS2 - 1))
                    # evacuate: acc2 = 2*psum - wsum2 ; stats; BN deferred
                    nc.vector.tensor_scalar(out=y2[ct][:], in0=pt[:, :], scalar1=2.0,
                                            scalar2=nws2_sb[:, ct:ct + 1],
                                            op0=Alu.mult, op1=Alu.add)
                s2 = st2_pool.tile([128, 4], f32)
                scr2 = ap_pool.tile([128, P2], f32)
                for ct in range(2):
                    nc.vector.tensor_scalar(out=scr2[:], in0=y2[ct][:], scalar1=0.0,
                                            scalar2=None, op0=Alu.add, op1=Alu.add,
                                            accum_out=s2[:, ct:ct + 1])
                    nc.scalar.activation(out=scr2[:], in_=y2[ct][:], func=Act.Square,
                                         accum_out=s2[:, 2 + ct:3 + ct])
                cc2 = allreduce([s2[:, 0:4]], [4], "bn2")
            st2 = sp.tile([128, 4], f32)
            nc.sync.dma_start(st2[:, 0:4],
                              cc2[0:1, 0:512].rearrange("one (p w) -> (one p) w", w=4))
            sc2, bi2 = bn_affine(st2[:, 0:4], gam2[:, 0:2], bet2[:, 0:2], 2,
                                 N_TOT * HO2 * W1, "bn2")
            for ct in range(2):
                nc.scalar.activation(out=act2[ct][:], in_=y2[ct][:], func=Act.Relu,
                                     scale=sc2[:, ct:ct + 1], bias=bi2[:, ct:ct + 1])
                for b in range(B3):
                    nc.vector.tensor_scalar(out=t3[b][ct][:], in0=act2[ct][:],
                                            scalar1=mid3_sb[:, b:b + 1], scalar2=None,
                                            op0=Alu.is_lt)

            # =================================================== layer 3
            with tc.tile_pool(name="ps3", bufs=3, space="PSUM") as ps3, \
                 tc.tile_pool(name="cs3p", bufs=1, space="PSUM") as cs3p:
                # column sums of act2 (both c-tiles) -> cs2 [1, P2]
                pcs2 = cs3p.tile([1, P2], f32, tag="pcs2", name="pcs2")
                for ct in range(2):
                    nc.tensor.matmul(pcs2[0:1, :], lhsT=ones_kb[:, 0:1],
                                     rhs=act2[ct][:], start=(ct == 0), stop=(ct == 1))
                nc.vector.tensor_copy(cs2[0:1, :], pcs2[0:1, :])
                cs2v = cs2.rearrange("one (n h w) -> one n h w", n=N_LOC, h=HO2, w=W1)
                xs3v = xs3.rearrange("one (n h w) -> one n h w", n=N_LOC, h=HO3, w=WO3)
                first = True
                for tap in range(TAPS3):
                    ki, kj = divmod(tap, 2)
                    view = cs2v[0:1, :, ki:ki + 2 * HO3 - 1:2, kj:kj + WO3]
                    if first:
                        nc.vector.tensor_scalar(out=xs3v[:], in0=view, scalar1=0.0,
                                                scalar2=None, op0=Alu.add)
                        first = False
                    else:
                        nc.vector.tensor_tensor(out=xs3v[:], in0=xs3v[:],
                                                in1=view, op=Alu.add)
                for ct in range(3):
                    pt3 = ps3.tile([128, P3], f32, tag="pt3", name="pt3")
                    nc.tensor.matmul(pt3[:, :], lhsT=halves_row[0:1, :],
                                     rhs=xs3[0:1, :], start=True, stop=False)
                    for b in range(B3):
                        for cib in range(2):
                            tv3 = t3[b][cib].rearrange("p (n h w) -> p n h w",
                                                       n=N_LOC, h=HO2, w=W1)
                            for tap in range(TAPS3):
                                ki, kj = divmod(tap, 2)
                                lix = (((b * 2 + cib) * TAPS3 + tap) * C3 + ct * 128)
                                nc.tensor.matmul(
                                    pt3[:, :],
                                    lhsT=pl3_sb[:, lix:lix + 128],
                                    rhs=tv3[:, :, ki:ki + 2 * HO3 - 1:2, kj:kj + WO3],
                                    start=False,
                                    stop=(b == B3 - 1 and cib == 1
                                          and tap == TAPS3 - 1))
                    nc.vector.tensor_scalar(out=y3[ct][:], in0=pt3[:, :], scalar1=2.0,
                                            scalar2=nws3_sb[:, ct:ct + 1],
                                            op0=Alu.mult, op1=Alu.add)
                s3 = sp.tile([128, 6], f32)
                scr3 = ap_pool.tile([128, P3], f32)
                for ct in range(3):
                    nc.vector.tensor_scalar(out=scr3[:], in0=y3[ct][:], scalar1=0.0,
                                            scalar2=None, op0=Alu.add, op1=Alu.add,
                                            accum_out=s3[:, ct:ct + 1])
                    nc.scalar.activation(out=scr3[:], in_=y3[ct][:], func=Act.Square,
                                         accum_out=s3[:, 3 + ct:4 + ct])
                cc3 = allreduce([s3[:, 0:6]], [6], "bn3")
            st3 = sp.tile([128, 6], f32)
            nc.sync.dma_start(st3[:, 0:6],
                              cc3[0:1, 0:768].rearrange("one (p w) -> (one p) w", w=6))
            sc3, bi3 = bn_affine(st3[:, 0:6], gam3[:, 0:3], bet3[:, 0:3], 3,
                                 N_TOT * HO3 * WO3, "bn3")
            for ct in range(3):
                nc.scalar.activation(out=act3[:, ct * P3:(ct + 1) * P3],
                                     in_=y3[ct][:], func=Act.Relu,
                                     scale=sc3[:, ct:ct + 1], bias=bi3[:, ct:ct + 1])

            # =================================================== FC + LN + L2
            with tc.tile_pool(name="psfc", bufs=1, space="PSUM") as psfc_p, \
                 tc.tile_pool(name="fcp", bufs=2) as fcp:
                fcacc = sp.tile([128, 12], f32)
                for jj in range(6):
                    for n in range(N_LOC):
                        prod = fcp.tile([128, 3 * 42], f32, tag="prod", name="prod")
                        a3v = act3.rearrange("p (cb q) -> p cb q", cb=3)[:, :, n * 42:(n + 1) * 42]
                        wv = wfc_sb.rearrange("p (j cb q) -> p j cb q", j=6, cb=3)[:, jj]
                        nc.vector.scalar_tensor_tensor(
                            out=prod[:], in0=a3v, scalar=0.0, in1=wv,
                            op0=Alu.add, op1=Alu.mult,
                            accum_out=fcacc[:, jj * 2 + n:jj * 2 + n + 1])
                psfc = psfc_p.tile([1, 12], f32)
                nc.tensor.matmul(psfc[0:1, :], lhsT=ones_k[:, 0:1], rhs=fcacc[:],
                                 start=True, stop=True)
                h12 = sp.tile([1, 12], f32)
                h12v = h12.rearrange("one (j n) -> one j n", n=N_LOC)
                psv = psfc.rearrange("one (j n) -> one j n", n=N_LOC)
                for n in range(N_LOC):
                    nc.vector.tensor_tensor(out=h12v[:, :, n], in0=psv[:, :, n],
                                            in1=bfc_sb[:], op=Alu.add)
                # LN stats
                lnS = sp.tile([1, 1], f32)
                lnQ = sp.tile([1, 1], f32)
                scrl = sp.tile([1, 12], f32)
                nc.vector.tensor_scalar(out=scrl[:], in0=h12[:], scalar1=0.0,
                                        scalar2=None, op0=Alu.add, op1=Alu.add,
                                        accum_out=lnS[:])
                nc.scalar.activation(out=scrl[:], in_=h12[:], func=Act.Square,
                                     accum_out=lnQ[:])
                ccl = allreduce([lnS[:], lnQ[:]], [1, 1], "ln")
                stl = sp.tile([1, 2], f32)
                nc.gpsimd.dma_start(stl[:], ccl[0:1, 0:2])
                mu = sp.tile([1, 1], f32)
                qv = sp.tile([1, 1], f32)
                mu2 = sp.tile([1, 1], f32)
                tvl = sp.tile([1, 1], f32)
                sl_ = sp.tile([1, 1], f32)
                rl0 = sp.tile([1, 1], f32)
                rl0sq = sp.tile([1, 1], f32)
                avl = sp.tile([1, 1], f32)
                bvl = sp.tile([1, 1], f32)
                rl = sp.tile([1, 1], f32)
                inv_tot = 1.0 / (N_TOT * 6)
                nc.vector.tensor_scalar(out=mu[:], in0=stl[:, 0:1], scalar1=inv_tot,
                                        scalar2=None, op0=Alu.mult)
                nc.vector.tensor_scalar(out=qv[:], in0=stl[:, 1:2], scalar1=inv_tot,
                                        scalar2=None, op0=Alu.mult)
                nc.vector.tensor_tensor(out=mu2[:], in0=mu[:], in1=mu[:], op=Alu.mult)
                nc.vector.scalar_tensor_tensor(out=tvl[:], in0=qv[:], scalar=EPS_LN,
                                               in1=mu2[:], op0=Alu.add, op1=Alu.subtract)
                nc.scalar.activation(out=sl_[:], in_=tvl[:], func=Act.Sqrt)
                nc.vector.reciprocal(out=rl0[:], in_=sl_[:])
                nc.vector.tensor_tensor(out=rl0sq[:], in0=rl0[:], in1=rl0[:], op=Alu.mult)
                nc.vector.tensor_tensor(out=avl[:], in0=tvl[:], in1=rl0sq[:], op=Alu.mult)
                nc.vector.tensor_scalar(out=bvl[:], in0=avl[:], scalar1=-0.5,
                                        scalar2=1.5, op0=Alu.mult, op1=Alu.add)
                nc.vector.tensor_tensor(out=rl[:], in0=rl0[:], in1=bvl[:], op=Alu.mult)
                y12 = sp.tile([1, 12], f32)
                nc.vector.tensor_scalar(out=y12[:], in0=h12[:], scalar1=mu[:],
                                        scalar2=rl[:], op0=Alu.subtract, op1=Alu.mult)
                ysq = sp.tile([1, 12], f32)
                nc.scalar.activation(out=ysq[:], in_=y12[:], func=Act.Square)
                out12 = sp.tile([1, 12], f32)
                y12v = y12.rearrange("one (j n) -> one j n", n=N_LOC)
                ysqv = ysq.rearrange("one (j n) -> one j n", n=N_LOC)
                o12v = out12.rearrange("one (j n) -> one j n", n=N_LOC)
                for n in range(N_LOC):
                    nrm = sp.tile([1, 1], f32, name=f"nrm_{n}")
                    srt = sp.tile([1, 1], f32, name=f"srt_{n}")
                    mx = sp.tile([1, 1], f32, name=f"mx_{n}")
                    ivn = sp.tile([1, 1], f32, name=f"ivn_{n}")
                    scrn = sp.tile([1, 6], f32, name=f"scrn_{n}")
                    nc.vector.tensor_scalar(out=scrn[:], in0=ysqv[:, :, n], scalar1=0.0,
                                            scalar2=None, op0=Alu.add, op1=Alu.add,
                                            accum_out=nrm[:])
                    nc.scalar.activation(out=srt[:], in_=nrm[:], func=Act.Sqrt)
                    nc.vector.tensor_scalar(out=mx[:], in0=srt[:], scalar1=EPS_L2,
                                            scalar2=None, op0=Alu.max)
                    nc.vector.reciprocal(out=ivn[:], in_=mx[:])
                    nc.vector.tensor_scalar(out=o12v[:, :, n], in0=y12v[:, :, n],
                                            scalar1=ivn[:], scalar2=None, op0=Alu.mult)
                outnj = sp.tile([1, 12], f32)
                nc.vector.tensor_copy(
                    outnj.rearrange("one (n j) -> one n j", n=N_LOC),
                    out12.rearrange("one (j n) -> one n j", n=N_LOC))
                nc.gpsimd.dma_start(out_d, outnj[:])

    nc.compile()
    return nc


def _log_grid(wmax, B):
    u = np.linspace(0.0, 1.0, B + 1)
    return (wmax * (np.expm1(u * np.log1p(4)) / 4)).astype(np.float64)


def _prep_inputs(inputs):
    """Host-side reshapes of the full inputs into per-core in_maps."""
    import ml_dtypes
    x = np.asarray(inputs["x"], np.float32)
    w1 = np.asarray(inputs["w1"], np.float32)
    w2 = np.asarray(inputs["w2"], np.float32)
    w3 = np.asarray(inputs["w3"], np.float32)
    Wfc = np.asarray(inputs["Wfc"], np.float32)

    nw1t = (-w1[:, 0, :, 0].T).reshape(1, TAPS1 * C1).astype(
        ml_dtypes.bfloat16)                                            # [1, 6*128]
    nw1s = (-w1[:, 0, :, 0].astype(ml_dtypes.bfloat16).astype(np.float32)
            .sum(1)).reshape(128, 1).copy()                            # -wsum1 [128,1]

    # --- layer 2 planes: w2 (256c, 128ci, 6kh, 1kw)
    g2 = _log_grid(float(w2.max()) + 1e-6, B2)
    m2 = ((g2[:-1] + g2[1:]) / 2).astype(np.float32)                   # (B2,)
    wt2 = w2[:, :, :, 0].transpose(1, 2, 0)                            # (ci, tap, c)
    pl2 = np.clip(wt2[None] - g2[:B2, None, None, None],
                  0.0, (g2[1:] - g2[:-1])[:, None, None, None])        # (b, ci, tap, c)
    pl2 = np.ascontiguousarray(pl2.transpose(1, 0, 2, 3)).reshape(
        128, B2 * TAPS2 * C2).astype(ml_dtypes.float8_e4m3fn)
    nws2 = (-w2.reshape(C2, -1).sum(1)).reshape(2, 128).T.copy()       # [128, 2]

    # --- layer 3 planes: w3 (384c, 256ci, 6kh, 2kw) ; tap = kh*2 + kw
    g3 = _log_grid(float(w3.max()) + 1e-6, B3)
    m3 = ((g3[:-1] + g3[1:]) / 2).astype(np.float32)
    wt3 = w3.reshape(C3, 2, 128, TAPS3).transpose(2, 1, 3, 0)          # (ci, cib, tap, c)
    pl3 = np.clip(wt3[None] - g3[:B3, None, None, None, None],
                  0.0, (g3[1:] - g3[:-1])[:, None, None, None, None])  # (b,ci,cib,tap,c)
    pl3 = np.ascontiguousarray(pl3.transpose(1, 0, 2, 3, 4)).reshape(
        128, B3 * 2 * TAPS3 * C3).astype(ml_dtypes.float8_e4m3fn)
    nws3 = (-w3.reshape(C3, -1).sum(1)).reshape(3, 128).T.copy()       # [128, 3]

    # Wfc: (6, 16128) with k = c3*42 + ho*2 + wo -> [ci, (j, cb, howo)]
    wf = Wfc.reshape(6, 3, 128, 42)                                    # (j, cb, ci, howo)
    wfcp = np.ascontiguousarray(wf.transpose(2, 0, 1, 3)).reshape(128, 6 * 3 * 42)

    shared = {
        "nw1t": nw1t, "nw1s": nw1s, "pl2": pl2, "pl3": pl3,
        "nws2": nws2, "nws3": nws3,
        "mid2": np.tile(m2[None, :], (128, 1)),
        "mid3": np.tile(m3[None, :], (128, 1)),
        "wfcp": wfcp,
        "g1_d": np.asarray(inputs["g1"], np.float32),
        "b1_d": np.asarray(inputs["b1"], np.float32),
        "g2_d": np.asarray(inputs["g2"], np.float32),
        "b2_d": np.asarray(inputs["b2"], np.float32),
        "g3_d": np.asarray(inputs["g3"], np.float32),
        "b3_d": np.asarray(inputs["b3"], np.float32),
        "bfc_d": np.asarray(inputs["bfc"], np.float32),
    }
    in_maps = []
    for i in range(N_CORES):
        m = dict(shared)
        xi = np.ascontiguousarray(
            x[i * N_LOC:(i + 1) * N_LOC]).reshape(1, N_LOC * H1 * W1)
        xh = xi.astype(ml_dtypes.bfloat16)
        xl = (xi - xh.astype(np.float32)).astype(ml_dtypes.bfloat16)
        m["xh_in"] = xh
        m["xl_in"] = xl
        in_maps.append(m)
    return in_maps


def _run(inputs, trace=False):
    if "nc" not in _BUILD_CACHE:
        _BUILD_CACHE["nc"] = build_program()
    nc = _BUILD_CACHE["nc"]
    from concourse import bass_utils
    in_maps = _prep_inputs(inputs)
    res = bass_utils.run_bass_kernel_spmd(
        nc, in_maps, core_ids=list(range(N_CORES)), trace=trace)
    out = np.concatenate(
        [np.asarray(r["out"]).reshape(N_LOC, 6) for r in res.results], axis=0)
    return out, res


def kernel(**inputs):
    return _run(inputs, trace=False)[0]
